# revision 38
# baseline (speedup 1.0000x reference)
"""FAVOR+ causal (Performer) attention kernel for 8 Trainium2 NeuronCores.

Problem: nn_Attention_87230785782564
  B=2, L=4096, E=512, H=8, DH=64, M=256 (feature dim), EPS=1e-6.

Sharding: data-parallel over batch B and head-parallel over H.
  core c -> batch b = c // 4, heads {2*(c%4), 2*(c%4)+1}.
Each core computes a partial output (sum over its 2 heads of av @ Wo);
the host sums the 4 cores per batch (f32) and adds bo.

v3 design (all-bf16 matmuls, engine-balanced):
  - q/k projections 2-heads-stacked [128, L] bf16; per-head [66, L] tiles
    (64 data rows + bias_hi/bias_lo rows) assembled via SBUF DMAs.
  - ||.||^2 rows computed column-native: lhsT = squares, rhs = [128, 2]
    ones-block matrix -> [C, 2] per chunk; bias rows built with full-lane
    [C, 32] column ops, then transpose-DMA'd into qkT rows 64/65.
  - k stabilizer: global max of transposed dd via vector TT-max tree +
    AllGather collective launched early; q-side work overlaps it.
  - scan per chunk C=128, both heads merged in each PSUM tile:
    A' [C, 2C], numT [65, 2C] (row 0 = den; Vaug col 0 is ones),
    T_c -> one [C, 260] S-carry bf16 updated by a single vector add.
  - division by den via gpsimd row-broadcast + vector TT divide (no
    1-lane reciprocals); den row killed in the output projection by a
    zero row 0 in the [65, E] Wo tiles.
"""

import sys

if "/opt/trn_rl_repo" not in sys.path:
    sys.path.insert(0, "/opt/trn_rl_repo")

import math

import numpy as np

import concourse.bass as bass
import concourse.tile as tile
from concourse import bacc, mybir
from concourse import bass_isa
from concourse.bass_utils import run_bass_kernel_spmd

B, L, E, H, DH, M = 2, 4096, 512, 8, 64, 256
EPS = 1e-6
N_CORES = 8
C = 128          # scan chunk
LT = 512         # l-tile for feature matmuls
N_LT = L // LT   # 8
N_CH = L // C    # 32
CPL = LT // C    # chunks per l-tile = 4

DN = 1.0 / math.sqrt(math.sqrt(float(DH)))   # data normalizer
RATIO = 1.0 / math.sqrt(float(M))            # 1/16
LNR = math.log(RATIO)
EPSR = RATIO * EPS

F32 = mybir.dt.float32
F32R = mybir.dt.float32r
BF16 = mybir.dt.bfloat16
AXX = mybir.AxisListType.X
TT_ADD = mybir.AluOpType.add
TT_SUB = mybir.AluOpType.subtract
TT_MULT = mybir.AluOpType.mult
TT_MAX = mybir.AluOpType.max
TT_DIV = mybir.AluOpType.divide
EXP = mybir.ActivationFunctionType.Exp
COPY = mybir.ActivationFunctionType.Copy


def build_nc():
    nc = bacc.Bacc("TRN2", target_bir_lowering=False)

    xTb = nc.dram_tensor("xTb", [E, L], BF16, kind="ExternalInput")
    wq = nc.dram_tensor("wq", [E, 2 * DH], BF16, kind="ExternalInput")
    wk = nc.dram_tensor("wk", [E, 2 * DH], BF16, kind="ExternalInput")
    wv = nc.dram_tensor("wv", [E, 2 * DH], BF16, kind="ExternalInput")
    wo65 = nc.dram_tensor("wo65", [2 * (DH + 1), E], BF16, kind="ExternalInput")
    projT_aug = nc.dram_tensor("projT_aug", [DH + 2, M], BF16, kind="ExternalInput")
    onesneg = nc.dram_tensor("onesneg", [2 * DH, 2], F32R, kind="ExternalInput")
    umask = nc.dram_tensor("umask", [C, C], F32, kind="ExternalInput")
    eye = nc.dram_tensor("eye", [C, C], BF16, kind="ExternalInput")
    out = nc.dram_tensor("out", [2 * L, E], BF16, kind="ExternalOutput")
    outden = nc.dram_tensor("outden", [N_CH, 2 * C], BF16, kind="ExternalOutput")

    with tile.TileContext(nc) as tc:
        _body(tc, nc, xTb, wq, wk, wv, wo65, projT_aug, onesneg, umask, eye, out, outden)
    nc.finalize()
    return nc


def _body(tc, nc, xTb, wq, wk, wv, wo65, projT_aug, onesneg, umask, eye, out, outden):
    from contextlib import ExitStack

    with ExitStack() as top:
        cpool = top.enter_context(tc.tile_pool(name="consts", bufs=1))
        dram = top.enter_context(tc.tile_pool(name="dram", bufs=1, space="DRAM"))

        # ---- persistent constants / tensors (live through the scan) ----
        pt_aug = cpool.tile([DH + 2, M], BF16, tag="pt_aug", name="pt_aug")
        nc.sync.dma_start(pt_aug[:], projT_aug[:, :])
        U = cpool.tile([C, C], F32, tag="U", name="U")
        nc.sync.dma_start(U[:], umask[:, :])
        eye_sb = cpool.tile([C, C], BF16, tag="eye", name="eye")
        nc.sync.dma_start(eye_sb[:], eye[:, :])
        wo_sb = [
            cpool.tile([DH + 1, E], BF16, tag=f"wo{h}", name=f"wo{h}") for h in range(2)
        ]
        for h in range(2):
            nc.sync.dma_start(wo_sb[h][:], wo65[h * (DH + 1) : (h + 1) * (DH + 1), :])

        # qkT[(h, t)]: [66, L] bf16; rows 0-63 data, 64 bias_hi, 65 bias_lo
        qkT = {
            (h, t): cpool.tile([DH + 2, L], BF16, tag=f"{t}T{h}", name=f"{t}T{h}")
            for h in range(2)
            for t in ("q", "k")
        }
        # Vaug2: per chunk a [2, 65] block (head, [ones | V]); col 0 = ones
        Vaug2 = cpool.tile([C, N_CH * 2 * (DH + 1)], BF16, tag="Vaug2", name="Vaug2")

        def vaug(ch, h):
            base = (2 * ch + h) * 65
            return Vaug2[:, base : base + 65]

        # prefix-state array: slot ch holds sum of T_c for c < ch.
        # slot layout: 4 blocks (2h+mh) of 65 cols; col 0 of a block = sden
        SLOT = 4 * (DH + 1)
        S_pref = cpool.tile([C, N_CH * SLOT], BF16, tag="S_pref", name="S_pref")
        nc.gpsimd.memset(S_pref[:, 0:SLOT], 0.0)

        # features (transposed): [128, 2L] per (h, t); cols mh*L + l
        feat = {
            (h, t): cpool.tile(
                [128, 2 * L], BF16, tag=f"f{t}{h}", name=f"f{t}{h}"
            )
            for h in range(2)
            for t in ("q", "k")
        }
        # k features natural: [C, M] per chunk
        kpn = [
            cpool.tile([C, N_CH * M], BF16, tag=f"kpn{h}", name=f"kpn{h}")
            for h in range(2)
        ]

        # ---------------- phase 1: projections + stabs + bias rows ----------
        with ExitStack() as p1:
            xpool = p1.enter_context(tc.tile_pool(name="xs", bufs=1))
            lpool = p1.enter_context(tc.tile_pool(name="ls", bufs=2))
            ps1 = p1.enter_context(tc.tile_pool(name="ps1", bufs=2, space="PSUM"))
            psd = p1.enter_context(tc.tile_pool(name="psd", bufs=2, space="PSUM"))
            psq_pool = p1.enter_context(tc.tile_pool(name="psq", bufs=1, space="PSUM"))
            psv = p1.enter_context(tc.tile_pool(name="psv", bufs=2, space="PSUM"))
            tiny = p1.enter_context(tc.tile_pool(name="tiny", bufs=1))
            psb = p1.enter_context(tc.tile_pool(name="psb", bufs=1, space="PSUM"))

            xts, wq_sb, wk_sb, wv_sb = [], [], [], []
            for et in range(4):
                t = xpool.tile([128, L], BF16, tag=f"xt{et}", name=f"xt{et}")
                xts.append(t)
            for et in range(4):
                a = xpool.tile([128, 2 * DH], BF16, tag=f"wq{et}", name=f"wq{et}")
                nc.sync.dma_start(a[:], wq[et * 128 : (et + 1) * 128, :])
                wq_sb.append(a)
                b = xpool.tile([128, 2 * DH], BF16, tag=f"wk{et}", name=f"wk{et}")
                nc.sync.dma_start(b[:], wk[et * 128 : (et + 1) * 128, :])
                wk_sb.append(b)
                v = xpool.tile([128, 2 * DH], BF16, tag=f"wv{et}", name=f"wv{et}")
                nc.sync.dma_start(v[:], wv[et * 128 : (et + 1) * 128, :])
                wv_sb.append(v)
            for xq in range(4):
                qs = slice(xq * (L // 4), (xq + 1) * (L // 4))
                for et in range(4):
                    nc.sync.dma_start(
                        xts[et][:, qs], xTb[et * 128 : (et + 1) * 128, qs]
                    )
            onn = xpool.tile([2 * DH, 2], F32R, tag="onn", name="onn")
            nc.sync.dma_start(onn[:], onesneg[:, :])

            # column-form stats: [C, N_CH] per head
            stabq_c = [
                xpool.tile([C, N_CH], F32, tag=f"sqc{h}", name=f"sqc{h}")
                for h in range(2)
            ]
            # sq columns: [C, 2*N_CH], (ch, h)-interleaved
            sq_col = {
                t: xpool.tile([C, 2 * N_CH], F32, tag=f"sqcol{t}", name=f"sqcol{t}")
                for t in ("q", "k")
            }
            macc = [
                xpool.tile([128, LT], F32, tag="macc0", name="macc0"),
            ]
            egb = xpool.tile([C, 1], F32, tag="egb", name="egb")

            # ---- q/k projections (k first: it gates the collective);
            # k-dd stab MMs interleaved per l-tile so the collective can
            # launch as soon as the k projection finishes.
            first = [True, True]
            for tname, wsb in (("k", wk_sb),):
                for lt in range(N_LT):
                    sl = slice(lt * LT, (lt + 1) * LT)
                    pp = ps1.tile([128, LT], F32, tag="pproj", name="pproj")
                    for et in range(4):
                        nc.tensor.matmul(
                            pp[:],
                            wsb[et][:],
                            xts[et][:, sl],
                            start=(et == 0),
                            stop=(et == 3),
                        )
                    qk2 = lpool.tile([128, LT], BF16, tag="qk2", name="qk2")
                    nc.scalar.copy(qk2[:], pp[:])
                    for h in range(2):
                        nc.sync.dma_start(
                            qkT[(h, tname)][0:DH, sl],
                            qk2[h * DH : (h + 1) * DH, :],
                        )
                    if tname == "k":
                        # k dd (transposed, no bias) -> running TT-max
                        for h in range(2):
                            for mh in range(2):
                                pd = psd.tile([128, LT], F32, tag="pdd", name="pdd")
                                nc.tensor.matmul(
                                    pd[:],
                                    pt_aug[0:DH, mh * 128 : (mh + 1) * 128],
                                    qkT[(h, "k")][0:DH, sl],
                                    start=True,
                                    stop=True,
                                )
                                i = 0
                                if first[i]:
                                    nc.vector.tensor_copy(macc[i][:], pd[:])
                                    first[i] = False
                                else:
                                    nc.vector.tensor_tensor(
                                        macc[i][:], macc[i][:], pd[:], op=TT_MAX
                                    )
                    # squares -> column-native -0.5*DN^2*||.||^2 via matmul
                    sqin = lpool.tile([128, LT], F32R, tag="sqin", name="sqin")
                    nc.gpsimd.tensor_tensor(sqin[:], qk2[:], qk2[:], op=TT_MULT)
                    psqc = psq_pool.tile([C, 2 * CPL], F32, tag="psqc", name="psqc")
                    for c4 in range(CPL):
                        nc.tensor.matmul(
                            psqc[:, 2 * c4 : 2 * c4 + 2],
                            sqin[:, c4 * C : (c4 + 1) * C],
                            onn[:],
                            start=True,
                            stop=True,
                        )
                    nc.vector.tensor_copy(
                        sq_col[tname][:, lt * 2 * CPL : (lt + 1) * 2 * CPL], psqc[:]
                    )
            kmax1 = tiny.tile([C, 1], F32, tag="kmax1", name="kmax1")
            nc.vector.reduce_max(kmax1[:], macc[0][:], axis=AXX)
            kmaxr = tiny.tile([C, 1], F32, tag="kmaxr", name="kmaxr")
            nc.gpsimd.partition_all_reduce(
                kmaxr[:], kmax1[:], channels=C, reduce_op=bass_isa.ReduceOp.max
            )
            cc_in = dram.tile([1, 1], F32)
            cc_out = dram.tile([N_CORES, 1], F32, addr_space="Shared")
            nc.sync.dma_start(cc_in[:], kmaxr[0:1, 0:1])
            nc.gpsimd.collective_compute(
                "AllGather",
                mybir.AluOpType.bypass,
                replica_groups=[list(range(N_CORES))],
                ins=[cc_in.opt()],
                outs=[cc_out.opt()],
            )

            # ---- q projection + squares (overlaps the collective) ----
            for tname, wsb in (("q", wq_sb),):
                for lt in range(N_LT):
                    sl = slice(lt * LT, (lt + 1) * LT)
                    pp = ps1.tile([128, LT], F32, tag="pproj", name="pproj")
                    for et in range(4):
                        nc.tensor.matmul(
                            pp[:],
                            wsb[et][:],
                            xts[et][:, sl],
                            start=(et == 0),
                            stop=(et == 3),
                        )
                    qk2 = lpool.tile([128, LT], BF16, tag="qk2", name="qk2")
                    nc.scalar.copy(qk2[:], pp[:])
                    for h in range(2):
                        nc.sync.dma_start(
                            qkT[(h, tname)][0:DH, sl],
                            qk2[h * DH : (h + 1) * DH, :],
                        )
                    sqin = lpool.tile([128, LT], F32R, tag="sqin", name="sqin")
                    nc.vector.tensor_tensor(sqin[:], qk2[:], qk2[:], op=TT_MULT)
                    psqc = psq_pool.tile([C, 2 * CPL], F32, tag="psqc", name="psqc")
                    for c4 in range(CPL):
                        nc.tensor.matmul(
                            psqc[:, 2 * c4 : 2 * c4 + 2],
                            sqin[:, c4 * C : (c4 + 1) * C],
                            onn[:],
                            start=True,
                            stop=True,
                        )
                    nc.vector.tensor_copy(
                        sq_col[tname][:, lt * 2 * CPL : (lt + 1) * 2 * CPL], psqc[:]
                    )
                    # q dd (natural) + per-row stabilizer for this l-tile
                    for h in range(2):
                        for cp in (2 * lt, 2 * lt + 1):
                            pdq = psd.tile([C, 2 * M], F32, tag="pdd", name="pddq")
                            for j in range(2):
                                ch = 2 * cp + j
                                nc.tensor.matmul(
                                    pdq[:, j * M : (j + 1) * M],
                                    qkT[(h, "q")][0:DH, ch * C : (ch + 1) * C],
                                    pt_aug[0:DH, :],
                                    start=True,
                                    stop=True,
                                )
                            p3d = pdq[:].rearrange("p (c q m) -> p c q m", q=2, m=M // 2)
                            t1 = lpool.tile([C, 2, M // 2], F32, tag="smt1", name="smt1")
                            nc.vector.tensor_copy(t1[:], p3d[:, :, 0, :])
                            nc.vector.tensor_tensor(t1[:], t1[:], p3d[:, :, 1, :], op=TT_MAX)
                            t2 = lpool.tile([C, 2, M // 4], F32, tag="smt2", name="smt2")
                            nc.vector.tensor_tensor(
                                t2[:], t1[:, :, 0 : M // 4], t1[:, :, M // 4 :], op=TT_MAX
                            )
                            nc.vector.reduce_max(
                                stabq_c[h][:, 2 * cp : 2 * cp + 2], t2[:], axis=AXX
                            )

            # ---- v projection (natural, per chunk) ----
            for ch in range(N_CH):
                pv = psv.tile([C, 2 * DH], F32, tag="pv", name="pv")
                for et in range(4):
                    nc.tensor.matmul(
                        pv[:],
                        xts[et][:, ch * C : (ch + 1) * C],
                        wv_sb[et][:],
                        start=(et == 0),
                        stop=(et == 3),
                    )
                dst = Vaug2[:, ch * 130 : (ch + 1) * 130].rearrange(
                    "p (h w) -> p h w", h=2
                )[:, :, 1:65]
                nc.vector.tensor_copy(
                    dst, pv[:].rearrange("p (h d) -> p h d", h=2)
                )
            ones_cols = Vaug2[:].rearrange(
                "p (c h w) -> p c h w", h=2, w=DH + 1
            )[:, :, :, 0:1]
            nc.gpsimd.memset(ones_cols, 1.0)

            # ---- read back the collective result (late: keeps the sync
            # DMA queue clear for the q-side DMAs emitted above) ----
            gmax_sb = tiny.tile([1, N_CORES], F32, tag="gmax_sb", name="gmax_sb")
            nc.sync.dma_start(gmax_sb[:], cc_out[:, :])
            gmax = tiny.tile([1, 1], F32, tag="gmax", name="gmax")
            nc.vector.reduce_max(gmax[:], gmax_sb[:], axis=AXX)
            # egb = EPSR * exp(gmax), broadcast down all partitions
            eg0 = tiny.tile([1, 1], F32, tag="eg0", name="eg0")
            nc.vector.tensor_scalar_add(eg0[:], gmax[:], math.log(EPSR))
            eg1 = tiny.tile([1, 1], F32, tag="eg1", name="eg1")
            nc.scalar.activation(eg1[:], eg0[:], EXP)
            nc.gpsimd.partition_broadcast(egb[:], eg1[:], channels=C)

            # ---- bias columns -> qkT rows 64 (hi) / 65 (lo) ----
            for tname in ("q", "k"):
                sc3 = sq_col[tname][:].rearrange("p (c h) -> p c h", h=2)
                for h in range(2):
                    tmp = tiny.tile([C, N_CH], F32, tag=f"tmp{tname}{h}", name="tmpb")
                    t2c = tiny.tile([C, N_CH], F32, tag=f"t2{tname}{h}", name="t2b")
                    bhc = tiny.tile([C, N_CH], BF16, tag=f"bh{tname}{h}", name="bhc")
                    blc = tiny.tile([C, N_CH], BF16, tag=f"bl{tname}{h}", name="blc")
                    if tname == "q":
                        # bias = sq - stab + LNR
                        nc.vector.tensor_tensor(
                            tmp[:], sc3[:, :, h], stabq_c[h][:], op=TT_SUB
                        )
                        nc.scalar.activation(bhc[:], tmp[:], COPY, bias=LNR)
                        nc.vector.tensor_tensor(t2c[:], tmp[:], bhc[:], op=TT_SUB)
                        nc.vector.tensor_scalar_add(blc[:], t2c[:], LNR)
                    else:
                        # bias = sq + LNR (gmax cancels in num/den; the EPS
                        # constant is scaled by e^gmax instead)
                        nc.scalar.activation(bhc[:], sc3[:, :, h], COPY, bias=LNR)
                        nc.vector.tensor_tensor(t2c[:], sc3[:, :, h], bhc[:], op=TT_SUB)
                        nc.vector.tensor_scalar_add(blc[:], t2c[:], LNR)
                    for row, bc in ((DH, bhc), (DH + 1, blc)):
                        ptr = psb.tile([N_CH, C], BF16, tag="ptr", name="ptr")
                        nc.tensor.transpose(ptr[:], bc[:], eye_sb[:])
                        brow = tiny.tile(
                            [N_CH, C], BF16, tag=f"brow{tname}{h}{row}", name="brow"
                        )
                        nc.vector.tensor_copy(brow[:], ptr[:])
                        nc.sync.dma_start(
                            qkT[(h, tname)][row : row + 1, :], brow[:]
                        )

        # ---------------- phases 2+3: features then scan ----------------
        with ExitStack() as p23:

            with ExitStack() as p2:
                psf = p2.enter_context(tc.tile_pool(name="psf", bufs=2, space="PSUM"))
                psS2 = p2.enter_context(tc.tile_pool(name="psS2", bufs=2, space="PSUM"))

                # q features first, then k (k gated on the collective)
                for tname in ("q", "k"):
                    for h in range(2):
                        for lt in range(N_LT):
                            sl = slice(lt * LT, (lt + 1) * LT)
                            pf = psf.tile([128, 2 * LT], F32, tag="pf", name="pf")
                            for mh in range(2):
                                nc.tensor.matmul(
                                    pf[:, mh * LT : (mh + 1) * LT],
                                    pt_aug[:, mh * 128 : (mh + 1) * 128],
                                    qkT[(h, tname)][:, sl],
                                    start=True,
                                    stop=True,
                                )
                            dst = feat[(h, tname)][:].rearrange(
                                "p (m l) -> p m l", m=2
                            )[:, :, sl]
                            nc.scalar.activation(dst, pf[:].rearrange(
                                "p (m l) -> p m l", m=2
                            ), EXP)
                            if tname == "q":
                                nc.vector.tensor_scalar_add(dst, dst, EPSR)
                            else:
                                nc.vector.tensor_scalar_add(dst, dst, egb[:])
                # k natural features (2 chunks per psum tile)
                for cp in range(N_CH // 2):
                    for h in range(2):
                        pfn = psf.tile([C, 2 * M], F32, tag="pfn", name="pfn")
                        for j in range(2):
                            ch = 2 * cp + j
                            nc.tensor.matmul(
                                pfn[:, j * M : (j + 1) * M],
                                qkT[(h, "k")][:, ch * C : (ch + 1) * C],
                                pt_aug[:, :],
                                start=True,
                                stop=True,
                            )
                        dst = kpn[h][:, 2 * cp * M : (2 * cp + 2) * M]
                        nc.scalar.activation(dst, pfn[:], EXP)
                        nc.vector.tensor_scalar_add(dst, dst, egb[:])
                    # T_c for these chunks + prefix accumulate into S_pref
                    for j in range(2):
                        ch = 2 * cp + j
                        if ch >= N_CH - 1:
                            continue
                        pS = psS2.tile([C, SLOT], F32, tag="pS", name="pS")
                        for h in range(2):
                            for mh in range(2):
                                b = 2 * h + mh
                                nc.tensor.matmul(
                                    pS[:, b * 65 : (b + 1) * 65],
                                    kpn[h][:, ch * M + mh * 128 : ch * M + (mh + 1) * 128],
                                    vaug(ch, h),
                                    start=True, stop=True,
                                )
                        nc.vector.tensor_tensor(
                            S_pref[:, (ch + 1) * SLOT : (ch + 2) * SLOT],
                            S_pref[:, ch * SLOT : (ch + 1) * SLOT],
                            pS[:],
                            op=TT_ADD,
                        )

            # ---------------- phase 3: scan ----------------
            with ExitStack() as p3:
                sc_sb = p3.enter_context(tc.tile_pool(name="sc_sb", bufs=4))
                rpool = p3.enter_context(tc.tile_pool(name="rpool", bufs=4))
                psa = p3.enter_context(tc.tile_pool(name="psa", bufs=1, space="PSUM"))
                psn = p3.enter_context(tc.tile_pool(name="psn", bufs=3, space="PSUM"))
                pso = p3.enter_context(tc.tile_pool(name="pso", bufs=2, space="PSUM"))

                for ch in range(N_CH):
                    cs = slice(ch * C, (ch + 1) * C)
                    pos = [
                        pso.tile([C, E], F32, tag=f"pout{h}", name=f"pout{h}")
                        for h in range(2)
                    ]
                    # A' for both heads in one psum tile [128, 2C]
                    pa = psa.tile([C, 2 * C], F32, tag="pA", name="pA")
                    for h in range(2):
                        hs = slice(h * C, (h + 1) * C)
                        for mh in range(2):
                            ms = slice(mh * L + ch * C, mh * L + (ch + 1) * C)
                            nc.tensor.matmul(
                                pa[:, hs],
                                feat[(h, "k")][:, ms],
                                feat[(h, "q")][:, ms],
                                start=(mh == 0),
                                stop=(mh == 1),
                            )
                    am = sc_sb.tile([C, 2 * C], BF16, tag="am", name="am")
                    nc.vector.tensor_tensor(
                        am[:].rearrange("p (h c) -> p h c", h=2),
                        pa[:].rearrange("p (h c) -> p h c", h=2),
                        U[:].rearrange("p (o c) -> p o c", o=1).broadcast_to([C, 2, C]),
                        op=TT_MULT,
                    )
                    # numT [65, 2C]; row 0 = den; both heads side by side
                    pn = psn.tile([DH + 1, 2 * C], F32, tag="pnum", name="pnum")
                    for h in range(2):
                        hs = slice(h * C, (h + 1) * C)
                        nc.tensor.matmul(
                            pn[:, hs], vaug(ch, h), am[:, hs], start=True, stop=False
                        )
                        for mh in range(2):
                            ms = slice(mh * L + ch * C, mh * L + (ch + 1) * C)
                            b = 2 * h + mh
                            nc.tensor.matmul(
                                pn[:, hs],
                                S_pref[:, ch * SLOT + b * 65 : ch * SLOT + (b + 1) * 65],
                                feat[(h, "q")][:, ms],
                                start=False, stop=(mh == 1),
                            )
                    # numT -> bf16 (den rides as row 0; divided on host)
                    nsc = sc_sb.tile([DH + 1, 2 * C], BF16, tag="nsc", name="nsc")
                    nc.vector.tensor_copy(nsc[:], pn[:])
                    nc.sync.dma_start(outden[ch : ch + 1, :], nsc[0:1, :])
                    # per-head out projection (division happens on host)
                    for h in range(2):
                        nc.tensor.matmul(
                            pos[h][:], nsc[:, h * C : (h + 1) * C], wo_sb[h][:],
                            start=True, stop=True,
                        )
                        osb = sc_sb.tile([C, E], BF16, tag=f"osb{h}", name="osb")
                        if (2 * ch + h) % 2 == 0:
                            nc.scalar.copy(osb[:], pos[h][:])
                        else:
                            nc.vector.tensor_copy(osb[:], pos[h][:])
                        nc.sync.dma_start(out[h * L + ch * C : h * L + (ch + 1) * C, :], osb[:])


_NC_CACHE = None


def build_in_maps(inputs):
    import ml_dtypes

    x = np.asarray(inputs["x"], np.float32)
    Wq = np.asarray(inputs["Wq"], np.float32)
    Wk = np.asarray(inputs["Wk"], np.float32)
    Wv = np.asarray(inputs["Wv"], np.float32)
    Wo = np.asarray(inputs["Wo"], np.float32)
    proj = np.asarray(inputs["proj"], np.float32)

    bf = ml_dtypes.bfloat16
    umask = np.triu(np.ones((C, C), np.float32))  # U[j, l] = 1 for j <= l
    pt_aug = np.ones((DH + 2, M), np.float32)
    pt_aug[0:DH, :] = (DN * proj).T
    pt_aug = pt_aug.astype(bf)
    onn = np.zeros((2 * DH, 2), np.float32)
    onn[0:DH, 0] = -0.5 * DN * DN
    onn[DH:, 1] = -0.5 * DN * DN
    eye = np.eye(C, dtype=np.float32).astype(bf)

    in_maps = []
    for c in range(N_CORES):
        b = c // 4
        h0 = 2 * (c % 4)
        wo65 = np.zeros((2 * (DH + 1), E), np.float32)
        wo65[1 : DH + 1] = Wo[h0]
        wo65[DH + 2 :] = Wo[h0 + 1]
        m = {
            "xTb": np.ascontiguousarray(x[b].T).astype(bf),
            "wq": np.ascontiguousarray(
                np.concatenate([Wq[:, h0, :], Wq[:, h0 + 1, :]], axis=1)
            ).astype(bf),
            "wk": np.ascontiguousarray(
                np.concatenate([Wk[:, h0, :], Wk[:, h0 + 1, :]], axis=1)
            ).astype(bf),
            "wv": np.ascontiguousarray(
                np.concatenate([Wv[:, h0, :], Wv[:, h0 + 1, :]], axis=1)
            ).astype(bf),
            "wo65": wo65.astype(bf),
            "projT_aug": pt_aug,
            "onesneg": onn,
            "umask": umask,
            "eye": eye,
        }
        in_maps.append(m)
    return in_maps


def kernel(**inputs):
    global _NC_CACHE
    if _NC_CACHE is None:
        _NC_CACHE = build_nc()
    nc = _NC_CACHE

    in_maps = build_in_maps(inputs)
    res = run_bass_kernel_spmd(nc, in_maps, core_ids=list(range(N_CORES)))

    bo = np.asarray(inputs["bo"], np.float32)
    outp = np.zeros((B, L, E), np.float32)
    for c in range(N_CORES):
        o = np.asarray(res.results[c]["out"], dtype=np.float32)  # [2L, E]
        den = np.asarray(res.results[c]["outden"], dtype=np.float32)  # [N_CH, 2C]
        for h in range(2):
            d = den[:, h * C : (h + 1) * C].reshape(L, 1)
            outp[c // 4] += o[h * L : (h + 1) * L, :] / d
    outp += bo[None, None, :]
    return outp
